# revision 15
# baseline (speedup 1.0000x reference)
"""Distributed multi-head attention kernel for 8 TRN2 NeuronCores.

Sharding: tensor-parallel over heads (2 heads/core). Per core: qkv projection
for its 128 features, attention for its 2 heads; the attention output is
exchanged with a per-query-block AllGather pipelined through the attention
phase, and the output projection is column-parallel (each core owns a 128-wide
slice of the output features, host concatenates).

Structure:
- q, k AND v are all projected in transposed [feat, rows] layout with the
  weight tile stationary (512-col matmuls); v is then moved to its natural
  [keys, feat] layout with background 128x128 DMA xbar transposes.
- QK uses row-split tile_position packing (two heads concurrent); PV uses
  column-split packing into one [128, 512] PSUM tile per block.
- Softmax denominators come from a bf16 pair/quad/hex DVE reduction tree over
  the exp tiles plus 4 all-ones matmuls per block into two [128, 512] PSUM
  halves; reciprocals via the DVE reciprocal_approx_fast custom op.
- The attention phase is emission-paced around the per-key-tile exp cadence:
  QK/exp of tile kt runs SHIFT tiles ahead of PV; projection rounds, v
  transposes and the per-block exchange/output work are emitted as fine-
  grained units popped between key tiles with deadline ordering.
- Exchange: after block b is normalized it is staged to DRAM and AllGathered
  ([128, 512] per core -> [1024, 512]); each core then runs its slice of the
  output projection for those 512 rows (8 k-chunk matmuls into PSUM), adds
  its bias slice and stores [128, 512] of the final output. All of this
  happens while later blocks' attention runs; only block 7's exchange chain
  (~10us) trails the attention phase.

Compute in bf16 on the PE array (f32 PSUM accumulation, f32 softmax
denominators/normalization). The host pre-transposes x to [dim, b*s] and
pre-casts x/wqkv/wo to bf16 as part of sharding/layout prep.
"""

import sys

sys.path.insert(0, "/opt/trn_rl_repo")

import ml_dtypes
import numpy as np

# Problem constants (hardcoded per harness contract)
B = 2
S = 2048
DIM = 1024
N_HEAD = 16
HD = 64  # head dim
SCALE = HD ** (-0.5)
R = B * S  # 4096 flattened rows
NCORES = 8
HPC = N_HEAD // NCORES  # 2 heads per core
FPC = HPC * HD  # 128 features per core
RPC = R // NCORES  # 512 rows per core (output row slice)

KT = DIM // 128  # 8 k-tiles over the model dim
NKT = S // 128  # 16 key tiles per sequence
NQB = S // 512  # 4 query blocks per sequence
NBLK = B * NQB  # 8 query blocks total
SHIFT = 3  # PV pipeline lag behind QK/exp

_CACHED = {}


def _build_graph():
    import concourse.mybir as mybir
    import concourse.tile as tile
    from concourse import bacc

    nc = bacc.Bacc(
        "TRN2",
        target_bir_lowering=False,
        debug=False,
        num_devices=NCORES,
    )
    return _build_body(nc, mybir, tile)


def _build_body(nc, mybir, tile):
    f32 = mybir.dt.float32
    bf16 = mybir.dt.bfloat16
    EXP = mybir.ActivationFunctionType.Exp

    xt = nc.dram_tensor("xt", [DIM, R], bf16, kind="ExternalInput").ap()
    wqkv = nc.dram_tensor("wqkv", [DIM, 3 * FPC], bf16, kind="ExternalInput").ap()
    bqkv = nc.dram_tensor("bqkv", [4, FPC], f32, kind="ExternalInput").ap()
    wo = nc.dram_tensor("wo", [DIM, FPC], bf16, kind="ExternalInput").ap()
    ident = nc.dram_tensor("ident", [128, 128], bf16, kind="ExternalInput").ap()
    out = nc.dram_tensor("out", [FPC, R], bf16, kind="ExternalOutput").ap()

    with tile.TileContext(nc) as tc:
        with (
            tc.tile_pool(name="glob", bufs=1) as glob,
            tc.tile_pool(name="dram", bufs=1, space="DRAM") as dram_pool,
        ):
            # ---------------- persistent tiles -------------------------
            ones128 = glob.tile([128, 128], bf16)
            nc.vector.memset(ones128[:], 1.0)
            ident_sb = glob.tile([128, 128], bf16)
            biases = glob.tile([128, 4], f32)  # q, k, v, o per-partition biases
            qT = glob.tile([128, R], bf16)
            kT = glob.tile([128, R], bf16)
            vT = glob.tile([128, R], bf16)  # [feat, keys]
            v_nat = glob.tile([128, R], bf16)  # [keys, 2h*64d] per 128-chunk

            ag_in = [
                dram_pool.tile([FPC, 512], bf16, name=f"ag_in{b}")
                for b in range(NBLK)
            ]
            ag_out = [
                dram_pool.tile(
                    [DIM, 512], bf16, name=f"ag_out{b}", addr_space="Shared"
                )
                for b in range(NBLK)
            ]

            # ---------------- phase 0: warm collective, weight DMAs ----
            # The first collective pays a ~40us comm-init barrier and its
            # completion wait parks on the issuing queue, so the warm
            # AllGather goes FIRST and gpsimd carries ONLY collectives;
            # weights ride the sync queue, x groups the scalar queue.
            warm_sb = glob.tile([1, 16], bf16)
            nc.vector.memset(warm_sb[:], 1.0)
            # touch Exp immediately so the ~1.3us ACT table load happens
            # during the initial DMA wait, not before the first real exp
            warm_act = glob.tile([1, 16], f32)
            nc.scalar.activation(warm_act[:], warm_sb[:], EXP)
            wq_all = glob.tile([128, KT * 3 * FPC], bf16)
            nc.sync.dma_start(
                out=wq_all[:].rearrange("p (k f) -> p k f", k=KT),
                in_=wqkv[:, :].rearrange("(k p) f -> p k f", k=KT),
            )
            wqkv_sb = [
                wq_all[:, k * 3 * FPC : (k + 1) * 3 * FPC] for k in range(KT)
            ]
            nc.sync.dma_start(
                out=biases[:, 0:4], in_=bqkv[:, :].rearrange("m p -> p m")
            )
            nc.sync.dma_start(out=ident_sb[:], in_=ident[:, :])
            wo_all = glob.tile([128, KT * FPC], bf16)
            nc.sync.dma_start(
                out=wo_all[:].rearrange("p (k f) -> p k f", k=KT),
                in_=wo[:, :].rearrange("(k p) f -> p k f", k=KT),
            )
            wo_sb = [wo_all[:, k * FPC : (k + 1) * FPC] for k in range(KT)]

            with tc.tile_pool(name="xTp", bufs=2) as xT_pool:

                def dma_group(g, eng=None):
                    """DMA one 1024-row group of xt; returns the 8 k-tiles."""
                    eng = eng or nc.sync
                    xg = []
                    for k in range(KT):
                        t = xT_pool.tile(
                            [128, 1024], bf16, name=f"xT_{k}", tag=f"xT{k}"
                        )
                        eng.dma_start(
                            out=t[:],
                            in_=xt[
                                k * 128 : (k + 1) * 128, g * 1024 : (g + 1) * 1024
                            ],
                        )
                        xg.append(t)
                    return xg

                PROJ_DST = None  # set below (needs qT/kT/vT in scope)

                def proj_mms(pp, xg, m, h, ks):
                    for k in ks:
                        nc.tensor.matmul(
                            pp[:],
                            lhsT=wqkv_sb[k][:, m * 128 : (m + 1) * 128],
                            rhs=xg[k][:, h * 512 : (h + 1) * 512],
                            start=(k == 0),
                            stop=(k == KT - 1),
                        )

                def proj_bias(pp, g, m, h):
                    col0 = g * 1024 + h * 512
                    dst = (qT, kT, vT)[m]
                    nc.vector.tensor_scalar_add(
                        out=dst[:, col0 : col0 + 512],
                        in0=pp[:],
                        scalar1=biases[:, m : m + 1],
                    )

                # -------- phase 1 prefix: just enough to start block 0 --
                xgs = {}
                with tc.tile_pool(name="pp1", bufs=2, space="PSUM") as pp1_pool:
                    xgs[0] = dma_group(0, nc.scalar)
                    xgs[1] = dma_group(1, nc.scalar)
                    # mini k-round: key tile 0 only, so the first QK/exp can
                    # issue several microseconds before the full rounds land
                    ppm = pp1_pool.tile([128, 128], f32, name="ppm", tag="pp")
                    for k in range(KT):
                        nc.tensor.matmul(
                            ppm[:],
                            lhsT=wqkv_sb[k][:, 128:256],
                            rhs=xgs[0][k][:, 0:128],
                            start=(k == 0),
                            stop=(k == KT - 1),
                        )
                    nc.vector.tensor_scalar_add(
                        out=kT[:, 0:128], in0=ppm[:], scalar1=biases[:, 1:2]
                    )
                    for m in (1, 0):  # k then q, rows 0-511
                        pp = pp1_pool.tile([128, 512], f32, name="pp", tag="pp")
                        proj_mms(pp, xgs[0], m, 0, range(KT))
                        proj_bias(pp, 0, m, 0)

                # -------- phase 2: attention + interleaved proj/exchange
                with (
                    tc.tile_pool(name="pstp", bufs=2, space="PSUM") as pst_pool,
                    tc.tile_pool(name="pop", bufs=1, space="PSUM") as po_pool,
                    tc.tile_pool(name="dwop", bufs=1, space="PSUM") as dwo_pool,
                    tc.tile_pool(name="trp", bufs=1, space="PSUM") as tr_pool,
                    tc.tile_pool(name="pp2", bufs=1, space="PSUM") as pp2_pool,
                    tc.tile_pool(name="ptp", bufs=12) as pt_pool,
                    tc.tile_pool(name="pairp", bufs=2) as pair_pool,
                    tc.tile_pool(name="quadp", bufs=2) as quad_pool,
                    tc.tile_pool(name="hexp", bufs=2) as hex_pool,
                    tc.tile_pool(name="recipp", bufs=2) as recip_pool,
                    tc.tile_pool(name="oTsp", bufs=2) as oTs_pool,
                    tc.tile_pool(name="gatp", bufs=2) as gat_pool,
                    tc.tile_pool(name="outp", bufs=2) as out_pool,
                ):
                    st = {"pp": None, "pending": None, "dn": None, "wo": None}

                    # ---- deferred work units (deadline-ordered) --------
                    def u_dma(g, eng):
                        return lambda: xgs.__setitem__(g, dma_group(g, eng))

                    def u_round_start(g, m, h, pool):
                        def f():
                            st["pp"] = pool.tile(
                                [128, 512], f32, name="pp", tag="pp"
                            )
                            proj_mms(st["pp"], xgs[g], m, h, range(2))

                        return f

                    def u_round_mid(g, m, h, ks):
                        return lambda: proj_mms(st["pp"], xgs[g], m, h, ks)

                    def u_round_end(g, m, h):
                        def f():
                            proj_mms(st["pp"], xgs[g], m, h, range(6, 8))
                            proj_bias(st["pp"], g, m, h)

                        return f

                    def round_units(m, g, h, pool):
                        return [
                            u_round_start(g, m, h, pool),
                            u_round_mid(g, m, h, range(2, 4)),
                            u_round_mid(g, m, h, range(4, 6)),
                            u_round_end(g, m, h),
                        ]

                    def u_tr(c):
                        def f():
                            t_ps = tr_pool.tile(
                                [128, 128], bf16, name="t_ps", tag="tr"
                            )
                            nc.tensor.transpose(
                                t_ps[:],
                                vT[:, c * 128 : (c + 1) * 128],
                                ident_sb[:],
                            )
                            nc.vector.tensor_copy(
                                out=v_nat[:, c * 128 : (c + 1) * 128],
                                in_=t_ps[:],
                            )

                        return f

                    def u_trs(cs):
                        return [u_tr(c) for c in cs]

                    # ---- per-block exchange / output projection --------
                    def u_gather(b, ks):
                        def f():
                            if f"gat{b}" not in st:
                                st[f"gat{b}"] = gat_pool.tile(
                                    [128, R], bf16, name="gat", tag="gat"
                                )
                            gat = st[f"gat{b}"]
                            for k in ks:
                                nc.sync.dma_start(
                                    out=gat[:, k * 512 : (k + 1) * 512],
                                    in_=ag_out[b][k * 128 : (k + 1) * 128, :],
                                )

                        return f

                    def u_wo_a(b):
                        def f():
                            gat = st[f"gat{b}"]
                            w_ps = dwo_pool.tile(
                                [128, 512], f32, name="w_ps", tag="dwo"
                            )
                            st["wo"] = w_ps
                            for k in range(4):
                                nc.tensor.matmul(
                                    w_ps[:],
                                    lhsT=wo_sb[k][:],
                                    rhs=gat[:, k * 512 : (k + 1) * 512],
                                    start=(k == 0),
                                    stop=False,
                                )

                        return f

                    def u_wo_b(b):
                        def f():
                            gat = st[f"gat{b}"]
                            w_ps = st["wo"]
                            for k in range(4, 8):
                                nc.tensor.matmul(
                                    w_ps[:],
                                    lhsT=wo_sb[k][:],
                                    rhs=gat[:, k * 512 : (k + 1) * 512],
                                    start=False,
                                    stop=(k == KT - 1),
                                )

                        return f

                    def u_wo_c(b):
                        def f():
                            w_ps = st["wo"]
                            o_sb = out_pool.tile(
                                [128, 512], bf16, name="o_sb", tag="o_sb"
                            )
                            nc.vector.tensor_scalar_add(
                                out=o_sb[:], in0=w_ps[:], scalar1=biases[:, 3:4]
                            )
                            nc.sync.dma_start(
                                out=out[:, b * 512 : (b + 1) * 512], in_=o_sb[:]
                            )
                            st.pop(f"gat{b}")

                        return f

                    def gat_units(b):
                        return [u_gather(b, range(0, 4)), u_gather(b, range(4, 8))]

                    def wo_units(b):
                        return [u_wo_a(b), u_wo_b(b), u_wo_c(b)]

                    p2 = pp2_pool
                    units = []
                    # block 0 needs all of batch-0 k, v (chunk c by kt c+3);
                    # block 1 needs q(0,1) at kt16.
                    units += round_units(2, 0, 0, p2) + u_trs([0, 1, 2, 3])
                    units += round_units(1, 0, 1, p2)  # k rows 512-1023 (kt4)
                    units += round_units(2, 0, 1, p2) + u_trs([4, 5, 6, 7])
                    units += round_units(1, 1, 0, p2)  # k rows 1024-1535 (kt8)
                    units += round_units(2, 1, 0, p2) + u_trs([8, 9, 10, 11])
                    units += round_units(1, 1, 1, p2)  # k rows 1536-2047 (kt12)
                    units += round_units(2, 1, 1, p2) + u_trs([12, 13, 14, 15])
                    units += round_units(0, 0, 1, p2)  # q for block 1
                    # popped during blocks 1-3: batch 1 k/v, q for blocks 2-4
                    units += round_units(0, 1, 0, p2)  # q for block 2
                    units += [u_dma(2, nc.sync)]
                    units += round_units(1, 2, 0, p2) + round_units(1, 2, 1, p2)
                    units += round_units(2, 2, 0, p2) + u_trs([16, 17, 18, 19])
                    units += round_units(0, 1, 1, p2)  # q for block 3
                    units += [u_dma(3, nc.sync)]
                    units += round_units(2, 2, 1, p2) + u_trs([20, 21, 22, 23])
                    units += round_units(1, 3, 0, p2) + round_units(1, 3, 1, p2)
                    units += round_units(0, 2, 0, p2)  # q for block 4
                    # popped during blocks 4-5 (v chunks 24-31 are due by
                    # block 4 kt15; q rounds one block ahead of use):
                    late_units = (
                        round_units(2, 3, 0, p2)
                        + u_trs([24, 25, 26, 27])
                        + round_units(2, 3, 1, p2)
                        + u_trs([28, 29, 30, 31])
                        + round_units(0, 2, 1, p2)  # q block 5
                        + round_units(0, 3, 0, p2)  # q block 6
                        + round_units(0, 3, 1, p2)  # q block 7
                    )
                    units.reverse()
                    late_units.reverse()
                    gath = {b: gat_units(b) for b in range(NBLK)}
                    wos = {b: wo_units(b) for b in range(NBLK)}

                    def emit_pv(blk, kt, pts, po, tree):
                        b = blk // NQB
                        off = (b * NKT + kt) * 128
                        pt = pts[kt]
                        nc.tensor.matmul(
                            po[0:64, :],
                            lhsT=v_nat[:, off : off + 64],
                            rhs=pt[:, 0:512],
                            start=(kt == 0),
                            stop=(kt == NKT - 1),
                            tile_position=(0, 0),
                        )
                        nc.tensor.matmul(
                            po[64:128, :],
                            lhsT=v_nat[:, off + 64 : off + 128],
                            rhs=pt[:, 512:1024],
                            start=(kt == 0),
                            stop=(kt == NKT - 1),
                            tile_position=(0, 64),
                        )
                        # bf16 reduction tree toward the denominators
                        if kt % 2 == 1:
                            pr = pair_pool.tile(
                                [128, 1024], bf16, name="pair", tag="pair"
                            )
                            nc.vector.tensor_add(
                                out=pr[:], in0=pts[kt - 1][:], in1=pt[:]
                            )
                            tree["pair"].append(pr)
                        if kt % 4 == 3:
                            qd = quad_pool.tile(
                                [128, 1024], bf16, name="quad", tag="quad"
                            )
                            nc.vector.tensor_add(
                                out=qd[:],
                                in0=tree["pair"][-2][:],
                                in1=tree["pair"][-1][:],
                            )
                            tree["quad"].append(qd)
                        if kt % 8 == 7:
                            hx = hex_pool.tile(
                                [128, 1024], bf16, name="hex", tag="hex"
                            )
                            nc.vector.tensor_add(
                                out=hx[:],
                                in0=tree["quad"][-2][:],
                                in1=tree["quad"][-1][:],
                            )
                            tree["hex"].append(hx)

                    def tail_a1(blk, pts, po, tree):
                        emit_pv(blk, NKT - 3, pts, po, tree)

                    def tail_a2(blk, pts, po, tree):
                        emit_pv(blk, NKT - 2, pts, po, tree)
                        emit_pv(blk, NKT - 1, pts, po, tree)

                    def den_half(tree, half):
                        dn = dwo_pool.tile([128, 512], f32, name="dn", tag="dwo")
                        c0 = half * 512
                        for hx, start in ((tree["hex"][0], True), (tree["hex"][1], False)):
                            nc.tensor.matmul(
                                dn[:],
                                lhsT=ones128[:],
                                rhs=hx[:, c0 : c0 + 512],
                                start=start,
                                stop=not start,
                            )
                        rc = recip_pool.tile(
                            [128, 512], f32, name="recip", tag="rc"
                        )
                        nc.vector.reciprocal_approx_fast(out=rc[:], in_=dn[:])
                        return rc

                    def tail_b(blk, pts, po, tree):
                        st["rcA"] = den_half(tree, 0)

                    def tail_c(blk, pts, po, tree):
                        rcB = den_half(tree, 1)
                        oTs = oTs_pool.tile([128, 512], bf16, name="oTs", tag="oTs")
                        nc.vector.tensor_mul(
                            out=oTs[0:64, :],
                            in0=po[0:64, :],
                            in1=st["rcA"][0:64, :],
                        )
                        nc.vector.tensor_mul(
                            out=oTs[64:128, :],
                            in0=po[64:128, :],
                            in1=rcB[64:128, :],
                        )
                        nc.sync.dma_start(out=ag_in[blk][:, :], in_=oTs[:])
                        nc.gpsimd.collective_compute(
                            "AllGather",
                            mybir.AluOpType.bypass,
                            replica_groups=[list(range(NCORES))],
                            ins=[ag_in[blk][:].opt()],
                            outs=[ag_out[blk][:].opt()],
                        )

                    for b in range(B):
                        for qb in range(NQB):
                            blk = b * NQB + qb
                            q0 = b * S + qb * 512
                            pts = []
                            tree = {"pair": [], "quad": [], "hex": []}
                            po = None
                            for kt in range(NKT):
                                k0 = b * S + kt * 128
                                pst = pst_pool.tile(
                                    [128, 1024], f32, name="pst", tag="st"
                                )
                                for hh in range(HPC):
                                    nc.tensor.matmul(
                                        pst[:, hh * 512 : (hh + 1) * 512],
                                        lhsT=kT[
                                            hh * 64 : (hh + 1) * 64, k0 : k0 + 128
                                        ],
                                        rhs=qT[
                                            hh * 64 : (hh + 1) * 64, q0 : q0 + 512
                                        ],
                                        start=True,
                                        stop=True,
                                        tile_position=(hh * 64, 0),
                                    )
                                pt = pt_pool.tile(
                                    [128, 1024], bf16, name="ptile", tag="pt"
                                )
                                nc.scalar.activation(
                                    pt[:], pst[:], EXP, scale=SCALE
                                )
                                pts.append(pt)
                                pend = st["pending"]
                                if kt == 0 and pend:
                                    tail_a1(*pend)
                                elif kt == 1 and pend:
                                    tail_a2(*pend)
                                elif kt == 2 and pend:
                                    tail_b(*pend)
                                elif kt == 3 and pend:
                                    tail_c(*pend)
                                    st["pending"] = None
                                if kt == SHIFT:
                                    po = po_pool.tile(
                                        [128, 512], f32, name="po", tag="po"
                                    )
                                if kt >= SHIFT:
                                    emit_pv(blk, kt - SHIFT, pts, po, tree)
                                # deadline-paced unit pops, kept away from the
                                # block-boundary key-tiles that carry the
                                # previous block's denominator/normalize work
                                npop = 0
                                if blk == 0:
                                    npop = 4 if kt < 2 else 3
                                elif blk < 4:
                                    npop = (
                                        1
                                        if kt in (1, 2, 3, 10, 11, 14, 15)
                                        else (2 if 4 <= kt < 10 else 0)
                                    )
                                elif blk < 7:
                                    npop = 2 if 2 <= kt < 9 else (1 if kt < 13 else 0)
                                for _ in range(npop):
                                    if units:
                                        units.pop()()
                                    elif late_units:
                                        late_units.pop()()
                                # pipelined exchange: block b's gathers pop
                                # two blocks later (kt 8-9, its AllGather has
                                # ~6 key-tiles of margin), the wo matmuls and
                                # bias+store three blocks later (kt 5,6,8 --
                                # after that block's own den tiles have
                                # released the shared PSUM bank)
                                if blk >= 2 and kt in (8, 9):
                                    eu = gath[blk - 2]
                                    if eu:
                                        eu.pop(0)()
                                if blk >= 3 and kt in (5, 6, 10):
                                    eu = wos[blk - 3]
                                    if eu:
                                        eu.pop(0)()
                                if blk == NBLK - 1 and kt in (11, 12, 13):
                                    eu = wos[NBLK - 3]
                                    if eu:
                                        eu.pop(0)()
                            st["pending"] = (blk, pts, po, tree)
                    # flush the last block
                    pend = st["pending"]
                    tail_a1(*pend)
                    tail_a2(*pend)
                    tail_b(*pend)
                    tail_c(*pend)
                    st["pending"] = None
                    while units:
                        units.pop()()
                    while late_units:
                        late_units.pop()()
                    for b in range(NBLK - 3, NBLK):
                        for u in gath[b] + wos[b]:
                            u()


    nc.compile()
    return nc


def _get_graph():
    if "nc" not in _CACHED:
        _CACHED["nc"] = _build_graph()
    return _CACHED["nc"]


def _make_in_maps(x, wqkv, bqkv, wo, bo):
    bf = ml_dtypes.bfloat16
    x2 = np.asarray(x, dtype=np.float32).reshape(R, DIM)
    xt = np.ascontiguousarray(x2.T.astype(bf))  # [dim, b*s] bf16
    wqkv = np.asarray(wqkv, dtype=np.float32)
    bqkv = np.asarray(bqkv, dtype=np.float32)
    wo = np.asarray(wo, dtype=np.float32)
    bo = np.asarray(bo, dtype=np.float32)
    ident = np.eye(128, dtype=bf)

    in_maps = []
    for c in range(NCORES):
        w_s = np.ascontiguousarray(
            np.concatenate(
                [
                    wqkv[:, c * FPC : (c + 1) * FPC],
                    wqkv[:, DIM + c * FPC : DIM + (c + 1) * FPC],
                    wqkv[:, 2 * DIM + c * FPC : 2 * DIM + (c + 1) * FPC],
                ],
                axis=1,
            ).astype(bf)
        )
        b_s = np.ascontiguousarray(
            np.stack(
                [
                    bqkv[c * FPC : (c + 1) * FPC],
                    bqkv[DIM + c * FPC : DIM + (c + 1) * FPC],
                    bqkv[2 * DIM + c * FPC : 2 * DIM + (c + 1) * FPC],
                    bo[c * FPC : (c + 1) * FPC],
                ],
                axis=0,
            )
        )
        wo_s = np.ascontiguousarray(
            wo[:, c * FPC : (c + 1) * FPC].astype(bf)
        )
        in_maps.append(
            {"xt": xt, "wqkv": w_s, "bqkv": b_s, "wo": wo_s, "ident": ident}
        )
    return in_maps


def _assemble(res):
    # core c's out is [128, 4096]: its 128 output features for all rows
    full = np.concatenate(
        [np.asarray(res.results[c]["out"]).T for c in range(NCORES)], axis=1
    )  # [4096, 1024]
    return np.ascontiguousarray(full.reshape(B, S, DIM)).astype(np.float32)


def kernel(x, wqkv, bqkv, wo, bo):
    from concourse.bass_utils import run_bass_kernel_spmd

    nc = _get_graph()
    in_maps = _make_in_maps(x, wqkv, bqkv, wo, bo)
    res = run_bass_kernel_spmd(nc, in_maps, core_ids=list(range(NCORES)))
    return _assemble(res)
